# revision 4
# baseline (speedup 1.0000x reference)
# InternLM2-7B decode-step paged attention on 8 Trainium2 NeuronCores, v2.
#
# Sharding (tensor-parallel, per the source hooks):
#   - wqkv column-sharded: core c gets q heads 4c..4c+3 and kv head c
#   - wo row-sharded: core c gets rows for q heads 4c..4c+3
#   - KV cache sharded along the kv-head dim: core c gets head c
#   - output projection partials summed on the host (the all-reduce)
#
# v2 design (vs the v1 row-major kernel):
#   - scores computed directly TRANSPOSED: per (seq, l-chunk) the K-tile
#     [d=128, l=128] is the PE stationary operand and q [d, 4 heads] the
#     moving one, so psum holds S^T[l, (s,h)] with no row-scatter DMAs.
#   - softmax without max-subtraction (scores here are bounded ~|s|<=10,
#     exp stays in f32 range; softmax is shift-invariant) -> exp straight
#     from psum into bf16 attnT, already in the layout the V matmul needs.
#     A multiplicative 0/1 mask (mz) zeroes invalid cache positions.
#   - denominators via ones-vector stationary matmuls over attnT chunks;
#     normalization by a rank-1 replicate matmul of 1/sums, one DVE mul
#     per seq.
#   - V accumulated transposed too: V-chunk [l=128, d=128] stationary,
#     attn [l, 4] moving -> psum [d, 4] per seq; no output transposes.
#   - new token handled as a rank-1 (K=1) outer-product matmul appended
#     to each seq's V accumulation group.
import os
import sys

for _p in (
    "/opt/trn_rl_repo",
    "/root/.axon_site",
    "/root/.axon_site/_ro/trn_rl_repo",
    "/root/.axon_site/_ro/pypackages",
):
    if os.path.isdir(_p) and _p not in sys.path:
        sys.path.append(_p)

import numpy as np
import ml_dtypes

BF16NP = ml_dtypes.bfloat16

import concourse.bass as bass
from concourse import bacc
import concourse.mybir as mybir
import concourse.tile as tile
from concourse.masks import make_identity

B = 32          # batch (decoding sequences)
H = 32          # query heads
KVH = 8         # kv heads
G = 4           # query heads per kv head (= per core)
HD = 128        # head dim
D = 4096        # model dim
W = (G + 2) * HD  # per-core qkv shard width = 768
L = 4096        # kv positions per sequence
NCH = L // 128  # 32 l-chunks of 128
CGK = 2         # l-chunks per kT DMA tile / psum slab
CGN = NCH // CGK  # 16 chunk groups
VSG = 2         # seqs per v DMA tile
KT_ = D // 128  # 32 contraction tiles for the qkv projection
BLOCK = 64
NBLK = 64
NCORES = 8
THETA = 1e6
R = G * B       # 128 row-cols (s-major: col = 4*s + h)

F32 = mybir.dt.float32
BF16 = mybir.dt.bfloat16
SCALE = 1.0 / float(np.sqrt(HD))


def _emit(nc, tc, hT, wq, wo, kTg, vv, mz, cs, pi, y):
    import contextlib

    Exp = mybir.ActivationFunctionType.Exp

    with contextlib.ExitStack() as ctx:
        singles = ctx.enter_context(tc.tile_pool(name="singles", bufs=1))
        wqp = ctx.enter_context(tc.tile_pool(name="wqp", bufs=2))
        ktp = ctx.enter_context(tc.tile_pool(name="ktp", bufs=4))
        vtp = ctx.enter_context(tc.tile_pool(name="vtp", bufs=3))
        stg = ctx.enter_context(tc.tile_pool(name="stg", bufs=3))
        # PSUM (8 banks): scp 3x1 + po 3x1 + sums 1 + rcps 1
        psA = ctx.enter_context(tc.tile_pool(name="psA", bufs=3, space="PSUM"))
        psB = ctx.enter_context(tc.tile_pool(name="psB", bufs=3, space="PSUM"))
        psD = ctx.enter_context(tc.tile_pool(name="psD", bufs=1, space="PSUM"))

        ident = singles.tile([128, 128], F32)
        make_identity(nc, ident)

        # ---- input loads (sync ring: hT/cs/wq/vv/y; scalar ring: mz/wo/kT)
        hT_sb = singles.tile([128, KT_, B], BF16)
        nc.sync.dma_start(hT_sb, hT)
        cst_sb = singles.tile([128, 4, B], F32)
        nc.sync.dma_start(cst_sb, cs)
        pi_sb = singles.tile([128, 128], F32)
        nc.sync.dma_start(pi_sb, pi)
        mz_sb = singles.tile([128, NCH, R], BF16)
        nc.scalar.dma_start(mz_sb, mz)
        wo_sb = singles.tile([128, G, D], BF16)
        nc.scalar.dma_start(wo_sb, wo.rearrange("(h p) n -> p h n", p=128))

        attnT = singles.tile([128, NCH, R], BF16)
        qT_buf = singles.tile([128, B, G], BF16)
        k_newT = singles.tile([128, B], F32)
        tmp_kq = singles.tile([128, B, G], F32)
        ones_bf = singles.tile([128, 1], BF16)
        nc.vector.memset(ones_bf, 1.0)
        ones_f = singles.tile([128, 1], F32)
        nc.vector.memset(ones_f, 1.0)
        ones_row = singles.tile([1, 128], F32)
        nc.vector.memset(ones_row, 1.0)
        pnew_row = singles.tile([1, R], BF16)
        vnew_row = singles.tile([1, B, HD], BF16)
        sums_f = singles.tile([1, R], F32)
        rc_row = singles.tile([1, R], F32)
        rc_sb = singles.tile([128, R], F32)
        outT_bf = singles.tile([128, B, G], BF16)

        # ---- fused QKV projection, wq stationary: qkvT[w, s] in psum.
        # Block wb of 128 w-rows: wb 0..3 = q heads, 4 = k, 5 = v.
        # Each block gets its own psum BANK: concurrently-open accumulation
        # groups must not share a bank.
        NB = W // 128
        qk_a = [psA.tile([128, 512], F32, tag="scp", name=f"qk_a{i}")
                for i in range(3)]
        qk_b = [psB.tile([128, 128], F32, tag="po", name=f"qk_b{i}")
                for i in range(3)]

        def qk_slot(wb):
            return (qk_a[wb][:, 0:B] if wb < 3 else qk_b[wb - 3][:, 0:B])

        for tq in range(KT_ // 4):
            wt = wqp.tile([128, 4, W], BF16, tag="wt")
            nc.sync.dma_start(wt, wq[:, tq * 4 : (tq + 1) * 4, :])
            for u in range(4):
                t = tq * 4 + u
                for wb in range(NB):
                    nc.tensor.matmul(
                        qk_slot(wb),
                        lhsT=wt[:, u, wb * 128 : (wb + 1) * 128],
                        rhs=hT_sb[:, t, :],
                        start=(t == 0), stop=(t == KT_ - 1))
        qkT_sb = singles.tile([128, NB * B], F32)
        for wb in range(NB):
            nc.vector.tensor_copy(qkT_sb[:, wb * B : (wb + 1) * B],
                                  qk_slot(wb))

        # ---- RoPE in transposed layout: out = cf.q + sf.(Pi^T q) ----
        qrot_ps = psA.tile([128, 512], F32, tag="scp")
        for wb in range(G + 1):
            nc.tensor.matmul(qrot_ps[:, wb * B : (wb + 1) * B],
                             lhsT=pi_sb[:, :],
                             rhs=qkT_sb[:, wb * B : (wb + 1) * B],
                             start=True, stop=True)
        for wb in range(G + 1):
            j = 0 if wb < G else 2  # scaled tables for q, unscaled for k
            cf = cst_sb[:, j, :]
            sf = cst_sb[:, j + 1, :]
            qv = qkT_sb[:, wb * B : (wb + 1) * B]
            rv = qrot_ps[:, wb * B : (wb + 1) * B]
            t1 = stg.tile([128, B], F32, tag="rt1")
            t2 = stg.tile([128, B], F32, tag="rt2")
            nc.vector.tensor_mul(t1, qv, cf)
            nc.vector.tensor_mul(t2, rv, sf)
            if wb < G:
                nc.vector.tensor_add(qT_buf[:, :, wb], t1, t2)
            else:
                nc.vector.tensor_add(k_newT, t1, t2)

        # ---- new-token v row: transpose v block back to [s, d] ----
        ps_t = psA.tile([128, 512], F32, tag="scp")
        nc.tensor.transpose(ps_t[:B, 0:HD], qkT_sb[:, G * B + B : NB * B],
                            ident[:, :])
        v_sbb = singles.tile([B, HD], BF16)
        nc.vector.tensor_copy(v_sbb, ps_t[:B, 0:HD])
        nc.gpsimd.dma_start(vnew_row[0:1, :, :], v_sbb[:, :])
        for s in range(B):
            nc.vector.tensor_scalar_mul(out=tmp_kq[:, s, :],
                                        in0=qT_buf[:, s, :],
                                        scalar1=k_newT[:, s : s + 1])
        pnew_ps = psB.tile([128, 128], F32, tag="po")
        nc.tensor.matmul(pnew_ps[0:1, 0:R], lhsT=ones_f[:, 0:1],
                         rhs=tmp_kq[:, :, :], start=True, stop=True)
        nc.scalar.activation(out=pnew_row[0:1, :], in_=pnew_ps[0:1, 0:R],
                             func=Exp)

        # ---- scores: attnT[l, c, (s,h)] = exp(S^T) * mask; sums ----
        sums_ps = psD.tile([1, R], F32, tag="sums")
        for cg in range(CGN):
            kt = ktp.tile([128, B, CGK, 128], BF16, tag="kt")
            nc.sync.dma_start(kt, kTg[cg, :, :, :, :])
            scp = psA.tile([128, 512], F32, tag="scp")
            for s in range(B):
                for u in range(CGK):
                    o = u * 128 + 4 * s
                    nc.tensor.matmul(scp[:, o : o + 4], lhsT=kt[:, s, u, :],
                                     rhs=qT_buf[:, s, :],
                                     start=True, stop=True)
            att = attnT[:, CGK * cg : CGK * cg + CGK, :]
            nc.scalar.activation(out=att.rearrange("p u r -> p (u r)"),
                                 in_=scp[:, 0 : CGK * 128], func=Exp)
            nc.vector.tensor_mul(att, att,
                                 mz_sb[:, CGK * cg : CGK * cg + CGK, :])
            for u in range(CGK):
                c = CGK * cg + u
                nc.tensor.matmul(sums_ps[0:1, :], lhsT=ones_bf[:, 0:1],
                                 rhs=attnT[:, c, :],
                                 start=(c == 0), stop=(c == NCH - 1))

        # ---- 1/(sums + p_new), replicated to all partitions ----
        nc.vector.tensor_copy(sums_f, sums_ps[0:1, :])
        nc.vector.tensor_add(sums_f, sums_f, pnew_row[0:1, :])
        nc.vector.reciprocal(rc_row, sums_f)
        rc_ps = psD.tile([128, R], F32, tag="rcps")
        nc.tensor.matmul(rc_ps[:, :], lhsT=ones_row[0:1, :],
                         rhs=rc_row[0:1, :], start=True, stop=True)
        nc.vector.tensor_copy(rc_sb, rc_ps[:, :])

        # ---- V phase: outT[d, (s,h)] = (V^T @ attn + v_new x p_new) * rc
        for sg in range(B // VSG):
            vt = vtp.tile([128, VSG, NCH, HD], BF16, tag="vt")
            nc.sync.dma_start(vt, vv[sg, :, :, :, :])
            for sl in range(VSG):
                s = sg * VSG + sl
                ps_o = psB.tile([128, 128], F32, tag="po")
                for c in range(NCH):
                    nc.tensor.matmul(ps_o[:, 0:4], lhsT=vt[:, sl, c, :],
                                     rhs=attnT[:, c, 4 * s : 4 * s + 4],
                                     start=(c == 0), stop=False)
                nc.tensor.matmul(ps_o[:, 0:4], lhsT=vnew_row[0:1, s, :],
                                 rhs=pnew_row[0:1, 4 * s : 4 * s + 4],
                                 start=False, stop=True)
                nc.vector.tensor_mul(outT_bf[:, s, :], ps_o[:, 0:4],
                                     rc_sb[:, 4 * s : 4 * s + 4])

        # ---- output projection partial: y = outT.T @ wo_shard ----
        for n in range(D // 512):
            ps_y = psA.tile([128, 512], F32, tag="scp")
            for h in range(G):
                nc.tensor.matmul(ps_y[:B, :], lhsT=outT_bf[:, :, h],
                                 rhs=wo_sb[:, h, n * 512 : (n + 1) * 512],
                                 start=(h == 0), stop=(h == G - 1))
            yst = stg.tile([B, 512], F32, tag="yst")
            nc.any.tensor_copy(yst, ps_y[:B, :])
            nc.scalar.dma_start(y[:, n * 512 : (n + 1) * 512], yst)


_NC_CACHE = None


def build_bass():
    global _NC_CACHE
    if _NC_CACHE is not None:
        return _NC_CACHE
    nc = bacc.Bacc("TRN2")
    hT = nc.dram_tensor("hT", [128, KT_, B], BF16, kind="ExternalInput")
    wq = nc.dram_tensor("wq", [128, KT_, W], BF16, kind="ExternalInput")
    wo = nc.dram_tensor("wo", [G * HD, D], BF16, kind="ExternalInput")
    kTg = nc.dram_tensor("kTg", [CGN, 128, B, CGK, 128], BF16,
                         kind="ExternalInput")
    vv = nc.dram_tensor("vv", [B // VSG, 128, VSG, NCH, HD], BF16,
                        kind="ExternalInput")
    mz = nc.dram_tensor("mz", [128, NCH, R], BF16, kind="ExternalInput")
    cs = nc.dram_tensor("cs", [128, 4, B], F32, kind="ExternalInput")
    pi = nc.dram_tensor("pi", [128, 128], F32, kind="ExternalInput")
    y = nc.dram_tensor("y", [B, D], F32, kind="ExternalOutput")
    with tile.TileContext(nc) as tc:
        _emit(nc, tc, hT[:, :, :], wq[:, :, :], wo[:, :], kTg[:, :, :, :, :],
              vv[:, :, :, :, :], mz[:, :, :], cs[:, :, :], pi[:, :], y[:, :])
    nc.finalize()
    _NC_CACHE = nc
    return nc


def make_host_inputs(hidden_states, wqkv, wo, k_cache, v_cache,
                     position_ids_1d, block_offsets, kv_seqlens):
    """Shard + preprocess full inputs into 8 per-core in_maps."""
    hidden_states = np.asarray(hidden_states, dtype=np.float32)
    wqkv = np.asarray(wqkv, dtype=np.float32)
    wo = np.asarray(wo, dtype=np.float32)
    k_cache = np.asarray(k_cache, dtype=np.float32)
    v_cache = np.asarray(v_cache, dtype=np.float32)
    position_ids_1d = np.asarray(position_ids_1d, dtype=np.int32)
    block_offsets = np.asarray(block_offsets, dtype=np.int32)
    kv_seqlens = np.asarray(kv_seqlens, dtype=np.int32)

    hTd = np.ascontiguousarray(
        hidden_states.T.reshape(KT_, 128, B).transpose(1, 0, 2)
    ).astype(BF16NP)  # [128, KT_, B]

    # RoPE tables in transposed layout: cst[p, j, s] with rows [cos;cos] /
    # [sin;sin]; j=0,1 pre-scaled (for q), j=2,3 unscaled (for k)
    inv_freq = (1.0 / (THETA ** (np.arange(0, HD, 2, dtype=np.float64) / HD)))
    ang = position_ids_1d.astype(np.float64)[:, None] * inv_freq[None, :]
    cf = np.concatenate([np.cos(ang), np.cos(ang)], axis=1).T  # [128, B]
    sf = np.concatenate([np.sin(ang), np.sin(ang)], axis=1).T
    cs_host = np.ascontiguousarray(np.stack(
        [cf * SCALE, sf * SCALE, cf, sf], axis=1)).astype(np.float32)

    # rotate-half permutation: (Pi^T q)[i] = -q[64+i], [64+i] = q[i]
    pi_host = np.zeros((128, 128), dtype=np.float32)
    for i in range(64):
        pi_host[64 + i, i] = -1.0
        pi_host[i, 64 + i] = 1.0

    # validity: cache position j valid iff j < seqlen-1 (cache row at
    # seqlen-1 is replaced by the new token, handled separately)
    j = np.arange(L, dtype=np.int64)[None, :]
    valid = (j < (kv_seqlens.astype(np.int64)[:, None] - 1))  # [B, L] bool

    # multiplicative bf16 mask in attnT layout [p, c, 4s+h]
    validT = valid.reshape(B, NCH, 128).transpose(2, 1, 0)  # [p, c, s]
    mz_host = np.ascontiguousarray(
        np.repeat(validT.astype(np.float32), G, axis=2)).astype(BF16NP)

    # paged gather: per-sequence kv via block table (a permutation of blocks)
    ident_blocks = np.array_equal(block_offsets.ravel(),
                                  np.arange(B * NBLK, dtype=np.int64))

    kx = np.moveaxis(k_cache, 2, 0)  # [KVH, NUM_BLOCKS, BLOCK, HD] (view)
    vx = np.moveaxis(v_cache, 2, 0)

    vmaskf = valid.astype(np.float32)[:, :, None]  # [B, L, 1]

    in_maps = []
    for c in range(NCORES):
        if ident_blocks:
            kg = kx[c].reshape(B, L, HD)
            vg = vx[c].reshape(B, L, HD)
        else:
            kg = kx[c][block_offsets].reshape(B, L, HD)
            vg = vx[c][block_offsets].reshape(B, L, HD)
        # kTg[cg, d, s, u, l] = K[s, (CGK*cg+u)*128+l, d]
        kTg_c = np.ascontiguousarray(
            kg.reshape(B, CGN, CGK, 128, HD).transpose(1, 4, 0, 2, 3)
        ).astype(BF16NP)
        # vv[sg, p, sl, c, d] = V[sg*VSG+sl, c*128+p, d], invalid pos zeroed
        vm = vg * vmaskf
        vv_c = np.ascontiguousarray(
            vm.reshape(B // VSG, VSG, NCH, 128, HD).transpose(0, 3, 1, 2, 4)
        ).astype(BF16NP)
        # wq[p, t, w] = wq_full[t*128+p, w] (partition-major for big descs)
        wq_c = np.ascontiguousarray(np.concatenate([
            wqkv[:, c * G * HD : (c + 1) * G * HD],
            wqkv[:, H * HD + c * HD : H * HD + (c + 1) * HD],
            wqkv[:, (H + KVH) * HD + c * HD : (H + KVH) * HD + (c + 1) * HD],
        ], axis=1).reshape(KT_, 128, W).transpose(1, 0, 2)).astype(BF16NP)
        wo_c = np.ascontiguousarray(
            wo[c * G * HD : (c + 1) * G * HD, :]).astype(BF16NP)  # [G*HD, D]
        in_maps.append(dict(hT=hTd, wq=wq_c, wo=wo_c, kTg=kTg_c, vv=vv_c,
                            mz=mz_host, cs=cs_host, pi=pi_host))
    return in_maps


def kernel(**inputs):
    from concourse.bass_utils import run_bass_kernel_spmd

    in_maps = make_host_inputs(
        inputs["hidden_states"], inputs["wqkv"], inputs["wo"],
        inputs["k_cache"], inputs["v_cache"], inputs["position_ids_1d"],
        inputs["block_offsets"], inputs["kv_seqlens"])
    nc = build_bass()
    res = run_bass_kernel_spmd(nc, in_maps, core_ids=list(range(NCORES)))
    y = np.zeros((B, D), dtype=np.float32)
    for r in res.results:
        y += np.asarray(r["y"], dtype=np.float32)
    return y
